# revision 3
# baseline (speedup 1.0000x reference)
"""Sparse Conv3d (3x3x3, torchsparse kmap) + BatchNorm + ReLU on 8 TRN2 NeuronCores.

Strategy (voxel/data parallel, per sharding hint):
  - Output voxels sharded across 8 cores (15000 rows each).
  - feats replicated in DRAM as 4 banks of 30000 rows (+1 zero row each) so
    gather indices fit int16 for the HW dma_gather path. Feature rows are
    stored fp16 padded to 128 channels (256B) so the gather can run in
    TRANSPOSE mode: the gather output is G^T (channels on partitions)
    directly, eliminating the per-chunk PE transpose + DVE copy of the
    previous version.
  - Center offset (k=13) is the identity map: its matmul operand is loaded
    directly from a per-core transposed feats slice (no gather).
  - Off-center offsets: host-compacted valid (src,dst) pairs; device does
    transpose-dma_gather (bank-major) -> matmul lhsT=G^T chunk, rhs=W_k ->
    dma_scatter_add (fp16 CCE) into fp16 SBUF accumulators (parity-split
    even/odd slots). Scatter calls are per-offset so destinations are unique
    within a call (duplicate dsts within one call lose updates on HW; across
    calls they accumulate correctly).
  - BN stats via PE matmuls (ones^T @ X for sums, X^T X diag for
    sum-squares), AllReduce [1,128] across the 8 cores, normalize + ReLU
    on-chip, fp16 output in a row-wrapped layout that the host unwraps.
"""

import sys
import os

for _p in ("/opt/trn_rl_repo", "/root/.axon_site/_ro/trn_rl_repo"):
    if os.path.isdir(_p) and _p not in sys.path:
        sys.path.insert(0, _p)

import numpy as np

N = 120000
CIN = 64
CPAD = 128                      # fp16 channel padding for 256B gather elems
COUT = 64
K = 27
CENTER = 13
EPS = 1e-5
NCORES = 8
NC_ROWS = N // NCORES          # 15000
BANK = 30000
NBANK = 4
ZROW = BANK                     # zero row index within a bank table
SLOTS = 118                     # ceil(15000/128); wrapped rows = 15104
WRAP_ROWS = SLOTS * 128         # 15104
TRASH = WRAP_ROWS - 1           # trash dst row 15103 (slot 117, partition 127)
HGRP = (SLOTS + 1) // 2         # 59 groups per parity


def _wrap16(idx):
    """Wrap an int stream into the [128, n/16] int16 layout dma_gather expects."""
    n = len(idx)
    assert n % 16 == 0
    w = np.ascontiguousarray(idx.reshape(n // 16, 16).T).astype(np.int16)
    return np.tile(w, (8, 1))


def _plan(nbr):
    """Host-side index preprocessing. Returns static chunk metadata (shared
    across cores) and per-core gather/scatter index tensors."""
    offs = [k for k in range(K) if k != CENTER]
    counts = np.zeros((NCORES, K, NBANK), np.int64)
    lists = {}
    for k in offs:
        v = nbr[k]
        for c in range(NCORES):
            seg = v[c * NC_ROWS:(c + 1) * NC_ROWS]
            val = np.nonzero(seg >= 0)[0]
            src = seg[val].astype(np.int64)
            bank = src // BANK
            for b in range(NBANK):
                m = bank == b
                lists[(c, k, b)] = (src[m] - b * BANK, val[m])
                counts[c, k, b] = int(m.sum())
    ckb = -(-counts.max(axis=0) // 128)  # [K, NBANK] chunks, 0 for CENTER row
    ckb[CENTER, :] = 0

    # gather stream: bank-major, offsets ascending inside each bank
    gbase = np.zeros(NBANK, np.int64)   # slot base per bank
    gslot = {}                          # (k, b) -> first gather slot (global)
    pos = 0
    sg_b = []
    for b in range(NBANK):
        gbase[b] = pos
        for k in offs:
            if ckb[k, b]:
                gslot[(k, b)] = pos
                pos += ckb[k, b]
        sg_b.append(pos - gbase[b])
    sg_total = pos

    # scatter stream: offset-major; chunk (k, b, j) -> scatter slot
    spos = {}
    p = 0
    ck_tot = {}
    for k in offs:
        ck = int(ckb[k].sum())
        ck_tot[k] = ck
        spos[k] = p
        p += ck
    ss_total = p

    gidx_cores, sidx_cores = [], []
    for c in range(NCORES):
        gstream = np.full(sg_total * 128, ZROW, np.int64)
        for b in range(NBANK):
            for k in offs:
                if not ckb[k, b]:
                    continue
                loc, _ = lists[(c, k, b)]
                s0 = gslot[(k, b)] * 128
                gstream[s0:s0 + len(loc)] = loc
        sstream = np.full(ss_total * 128, TRASH, np.int64)
        for k in offs:
            base = spos[k] * 128
            o = 0
            for b in range(NBANK):
                if not ckb[k, b]:
                    continue
                _, dst = lists[(c, k, b)]
                sstream[base + o:base + o + len(dst)] = dst
                o += ckb[k, b] * 128
        # wrap per call: gather call = per bank; scatter call = per offset
        gw = [
            _wrap16(gstream[gbase[b] * 128:(gbase[b] + sg_b[b]) * 128])
            for b in range(NBANK) if sg_b[b]
        ]
        sw = [
            _wrap16(sstream[spos[k] * 128:(spos[k] + ck_tot[k]) * 128])
            for k in offs if ck_tot[k]
        ]
        gidx_cores.append(np.concatenate(gw, axis=1))
        sidx_cores.append(np.concatenate(sw, axis=1))

    meta = dict(offs=offs, ckb=ckb, gbase=gbase, sg_b=sg_b, sg_total=sg_total,
                gslot=gslot, spos=spos, ck_tot=ck_tot, ss_total=ss_total)
    return meta, gidx_cores, sidx_cores


def _build_bass(meta, repeat=1):
    from concourse import mybir, bacc
    import concourse.tile as tile
    from concourse.masks import make_identity

    offs = meta["offs"]
    ckb = meta["ckb"]
    gbase = meta["gbase"]
    sg_b = meta["sg_b"]
    gslot = meta["gslot"]
    ck_tot = meta["ck_tot"]
    sg_total = meta["sg_total"]
    ss_total = meta["ss_total"]
    f32 = mybir.dt.float32
    f16 = mybir.dt.float16
    i16 = mybir.dt.int16

    nc = bacc.Bacc("TRN2", target_bir_lowering=False, debug=False,
                   num_devices=NCORES)
    feats4 = nc.dram_tensor("feats4", [NBANK * (BANK + 1), CPAD], f16,
                            kind="ExternalInput").ap()
    wmat = nc.dram_tensor("wmat", [CPAD, K * COUT], f16,
                          kind="ExternalInput").ap()
    ftc = nc.dram_tensor("ftc", [CIN, WRAP_ROWS], f16,
                         kind="ExternalInput").ap()
    gidx = nc.dram_tensor("gidx", [128, sg_total * 8], i16,
                          kind="ExternalInput").ap()
    sixd = nc.dram_tensor("sixd", [128, ss_total * 8], i16,
                          kind="ExternalInput").ap()
    gbeta = nc.dram_tensor("gbeta", [1, 128], f32, kind="ExternalInput").ap()
    oute = nc.dram_tensor("oute", [128, HGRP, COUT], f16,
                          kind="ExternalOutput").ap()
    outo = nc.dram_tensor("outo", [128, HGRP, COUT], f16,
                          kind="ExternalOutput").ap()

    with tile.TileContext(nc) as tc:
        with tc.tile_pool(name="sb", bufs=1) as pool, \
             tc.tile_pool(name="ps", bufs=2, space="PSUM") as psum, \
             tc.tile_pool(name="dram", bufs=1, space="DRAM") as dram:
            for _rep in range(repeat):
                ident = pool.tile([128, 128], f32)
                make_identity(nc, ident[:])
                ones16 = pool.tile([128, 1], f16)
                nc.vector.memset(ones16[:], 1.0)
                onesr = pool.tile([1, 128], f32)
                nc.vector.memset(onesr[:], 1.0)
                istack = pool.tile([128, COUT], f32)
                nc.vector.tensor_copy(out=istack[0:64, :], in_=ident[0:64, 0:64])
                nc.vector.tensor_copy(out=istack[64:128, :],
                                      in_=ident[64:128, 64:128])

                gix = pool.tile([128, sg_total * 8], i16)
                nc.sync.dma_start(out=gix[:], in_=gidx[:])
                six = pool.tile([128, ss_total * 8], i16)
                nc.sync.dma_start(out=six[:], in_=sixd[:])
                wsb = pool.tile([CPAD, K * COUT], f16)
                nc.sync.dma_start(out=wsb[:], in_=wmat[:])
                gb = pool.tile([1, 128], f32)
                nc.sync.dma_start(out=gb[:], in_=gbeta[:])

                # accumulators (SBUF resident, fp16), initialized by center pass
                ae = pool.tile([128, HGRP, COUT], f16)
                ao = pool.tile([128, HGRP, COUT], f16)

                # ---- transpose-mode gathers (bank-major stream) ----
                gtiles = {}
                for b in range(NBANK):
                    if not sg_b[b]:
                        continue
                    g = pool.tile([128, 1, sg_b[b] * 128], f16, tag=f"g{b}")
                    gtiles[b] = g
                    nc.gpsimd.dma_gather(
                        out_ap=g[:],
                        in_ap=feats4[(BANK + 1) * b:(BANK + 1) * (b + 1), :],
                        idxs_ap=gix[:, gbase[b] * 8:(gbase[b] + sg_b[b]) * 8],
                        num_idxs=sg_b[b] * 128, num_idxs_reg=sg_b[b] * 128,
                        elem_size=CPAD, transpose=True, single_packet=False)

                # ---- center pass: ftc slices -> matmul -> init ae/ao ----
                piece_chunks = [30, 30, 30, 28]
                wc = wsb[0:CIN, CENTER * COUT:(CENTER + 1) * COUT]
                jglob = 0
                for pc_i, pch in enumerate(piece_chunks):
                    fpc = pool.tile([CIN, pch * 128], f16, tag="ftc", bufs=2)
                    nc.sync.dma_start(
                        out=fpc[:], in_=ftc[:, jglob * 128:(jglob + pch) * 128])
                    for j0 in range(0, pch, 8):
                        jn = min(8, pch - j0)
                        pe = psum.tile([128, 4, COUT], f32, tag="pcE")
                        po = psum.tile([128, 4, COUT], f32, tag="pcO")
                        ne = no = 0
                        for j in range(j0, j0 + jn):
                            gj = jglob + j
                            lhsT = fpc[:, j * 128:(j + 1) * 128]
                            if gj % 2 == 0:
                                out_ap = pe[:, ne, :]; ne += 1
                            else:
                                out_ap = po[:, no, :]; no += 1
                            nc.tensor.matmul(out=out_ap, lhsT=lhsT, rhs=wc,
                                             start=True, stop=True)
                        ge0 = (jglob + j0) // 2
                        go0 = (jglob + j0) // 2
                        if ne:
                            nc.vector.tensor_copy(out=ae[:, ge0:ge0 + ne, :],
                                                  in_=pe[:, :ne, :])
                        if no:
                            nc.vector.tensor_copy(out=ao[:, go0:go0 + no, :],
                                                  in_=po[:, :no, :])
                    jglob += pch

                # ---- off-center: G^T chunk matmul -> scatter-add ----
                for k in offs:
                    ck = ck_tot[k]
                    if not ck:
                        continue
                    y = pool.tile([128, ck, COUT], f16, tag="y", bufs=3)
                    wk = wsb[:, k * COUT:(k + 1) * COUT]
                    # chunk list for this offset in scatter order
                    chunks = []
                    for b in range(NBANK):
                        for j in range(ckb[k, b]):
                            chunks.append(gslot[(k, b)] - gbase[b] + j
                                          + (b << 20))  # encode bank
                    for i0 in range(0, ck, 8):
                        inb = min(8, ck - i0)
                        py = psum.tile([128, 8, COUT], f32, tag="py")
                        for q in range(inb):
                            enc = chunks[i0 + q]
                            b, slot = enc >> 20, enc & ((1 << 20) - 1)
                            gT = gtiles[b][:, 0, slot * 128:(slot + 1) * 128]
                            nc.tensor.matmul(out=py[:, q, :], lhsT=gT, rhs=wk,
                                             start=True, stop=True)
                        nc.vector.tensor_copy(out=y[:, i0:i0 + inb, :],
                                              in_=py[:, :inb, :])
                    nc.gpsimd.dma_scatter_add(
                        out_ap=ae[:], in_ap=y[:, :, :],
                        idxs_ap=six[:, meta["spos"][k] * 8:(meta["spos"][k] + ck) * 8],
                        num_idxs=ck * 128, num_idxs_reg=ck * 128, elem_size=COUT,
                        sbuf_tokens_per_rank=128, parity_reg=0, out_ap_other=ao[:],
                        single_packet=False)

                # ---- zero the trash region (rows 15072..15103 incl. TRASH) ----
                # other pad rows (15000..15071) only ever receive center zeros
                nc.vector.memset(ao[96:128, 58, :], 0.0)

                # ---- stats: sums + sum-squares over all rows ----
                pcov = psum.tile([128, 128], f32, tag="py")
                cov_ins = []
                for t in (ae, ao):
                    for g0 in range(0, HGRP - 1, 2):
                        cov_ins.append(t[:, g0:g0 + 2, :])
                    cov_ins.append(t[:, HGRP - 1:HGRP, :])
                for i, ap in enumerate(cov_ins):
                    w = ap.shape[1] * COUT
                    nc.tensor.matmul(out=pcov[0:w, 0:w], lhsT=ap, rhs=ap,
                                     start=(i == 0), stop=(i == len(cov_ins) - 1))
                psumr = psum.tile([1, 512], f32, tag="pcE")
                sum_ins = []
                for t in (ae, ao):
                    for g0 in range(0, HGRP, 8):
                        gn = min(8, HGRP - g0)
                        sum_ins.append(t[:, g0:g0 + gn, :])
                for i, ap in enumerate(sum_ins):
                    w = ap.shape[1] * COUT
                    nc.tensor.matmul(out=psumr[:, 0:w], lhsT=ones16[:], rhs=ap,
                                     start=(i == 0), stop=(i == len(sum_ins) - 1))
                tmpc = pool.tile([128, 128], f32)
                nc.vector.tensor_mul(out=tmpc[:], in0=pcov[:], in1=ident[:])
                diagc = pool.tile([128, 1], f32)
                nc.vector.tensor_reduce(out=diagc[:], in_=tmpc[:],
                                        axis=mybir.AxisListType.X,
                                        op=mybir.AluOpType.add)
                psq = psum.tile([1, COUT], f32, tag="pt")
                nc.tensor.matmul(out=psq[:], lhsT=diagc[:], rhs=istack[:],
                                 start=True, stop=True)
                ssum = pool.tile([1, 512], f32)
                nc.vector.tensor_copy(out=ssum[:], in_=psumr[:])
                nc.vector.tensor_add(out=ssum[:, 0:256], in0=ssum[:, 0:256],
                                     in1=ssum[:, 256:512])
                nc.vector.tensor_add(out=ssum[:, 0:128], in0=ssum[:, 0:128],
                                     in1=ssum[:, 128:256])
                nc.vector.tensor_add(out=ssum[:, 0:64], in0=ssum[:, 0:64],
                                     in1=ssum[:, 64:128])
                stats = pool.tile([1, 128], f32)
                nc.vector.tensor_copy(out=stats[:, 0:64], in_=ssum[:, 0:64])
                nc.vector.tensor_copy(out=stats[:, 64:128], in_=psq[:])

                # ---- AllReduce over 8 cores ----
                cin_d = dram.tile([1, 128], f32)
                cout_d = dram.tile([1, 128], f32)
                nc.sync.dma_start(out=cin_d[:], in_=stats[:])
                nc.gpsimd.collective_compute(
                    "AllReduce", mybir.AluOpType.add,
                    replica_groups=[list(range(NCORES))],
                    ins=[cin_d.opt()], outs=[cout_d.opt()])
                red = pool.tile([1, 128], f32)
                nc.sync.dma_start(out=red[:], in_=cout_d[:])

                # ---- affine params ----
                mean = pool.tile([1, COUT], f32)
                nc.vector.tensor_scalar_mul(out=mean[:], in0=red[:, 0:64],
                                            scalar1=1.0 / N)
                ex2 = pool.tile([1, COUT], f32)
                nc.vector.tensor_scalar_mul(out=ex2[:], in0=red[:, 64:128],
                                            scalar1=1.0 / N)
                var = pool.tile([1, COUT], f32)
                nc.vector.tensor_mul(out=var[:], in0=mean[:], in1=mean[:])
                nc.vector.tensor_sub(out=var[:], in0=ex2[:], in1=var[:])
                nc.vector.tensor_scalar_add(out=var[:], in0=var[:], scalar1=EPS)
                std = pool.tile([1, COUT], f32)
                nc.scalar.sqrt(out=std[:], in_=var[:])
                rstd = pool.tile([1, COUT], f32)
                nc.vector.reciprocal(out=rstd[:], in_=std[:])
                scl = pool.tile([1, COUT], f32)
                nc.vector.tensor_mul(out=scl[:], in0=gb[:, 0:64], in1=rstd[:])
                bia = pool.tile([1, COUT], f32)
                nc.vector.tensor_mul(out=bia[:], in0=mean[:], in1=scl[:])
                nc.vector.tensor_sub(out=bia[:], in0=gb[:, 64:128], in1=bia[:])

                # broadcast to [128, 8, 64] fp16 (pattern repeats every 64 cols)
                pbs = psum.tile([128, COUT], f32, tag="pt")
                nc.tensor.matmul(out=pbs[:], lhsT=onesr[:], rhs=scl[:],
                                 start=True, stop=True)
                s8 = pool.tile([128, 8, COUT], f16)
                nc.vector.tensor_copy(out=s8[:, 0, :], in_=pbs[:])
                pbb = psum.tile([128, COUT], f32, tag="pt")
                nc.tensor.matmul(out=pbb[:], lhsT=onesr[:], rhs=bia[:],
                                 start=True, stop=True)
                b8 = pool.tile([128, 8, COUT], f16)
                nc.vector.tensor_copy(out=b8[:, 0, :], in_=pbb[:])
                for t8 in (s8, b8):
                    nc.vector.tensor_copy(out=t8[:, 1:2, :], in_=t8[:, 0:1, :])
                    nc.vector.tensor_copy(out=t8[:, 2:4, :], in_=t8[:, 0:2, :])
                    nc.vector.tensor_copy(out=t8[:, 4:8, :], in_=t8[:, 0:4, :])

                # ---- normalize + relu in place, then write out ----
                for t in (ae, ao):
                    for g0 in range(0, HGRP, 8):
                        gn = min(8, HGRP - g0)
                        sl = t[:, g0:g0 + gn, :]
                        nc.vector.tensor_mul(out=sl, in0=sl, in1=s8[:, 0:gn, :])
                        nc.vector.tensor_add(out=sl, in0=sl, in1=b8[:, 0:gn, :])
                        nc.vector.tensor_scalar_max(out=sl, in0=sl, scalar1=0.0)
                nc.sync.dma_start(out=oute[:], in_=ae[:, :, :])
                nc.sync.dma_start(out=outo[:], in_=ao[:, :, :])

    nc.compile()
    return nc


def _host_tensors(feats, weight, gamma, beta):
    feats = np.ascontiguousarray(np.asarray(feats, dtype=np.float32))
    weight = np.asarray(weight, dtype=np.float32)
    f4 = np.zeros((NBANK * (BANK + 1), CPAD), np.float16)
    for b in range(NBANK):
        f4[b * (BANK + 1):b * (BANK + 1) + BANK, :CIN] = \
            feats[b * BANK:(b + 1) * BANK]
    wm = np.zeros((CPAD, K * COUT), np.float16)
    wm[:CIN, :] = weight.transpose(1, 0, 2).reshape(CIN, K * COUT)
    gbv = np.zeros((1, 128), np.float32)
    gbv[0, 0:64] = np.asarray(gamma, np.float32)
    gbv[0, 64:128] = np.asarray(beta, np.float32)
    ftcs = []
    for c in range(NCORES):
        t = np.zeros((CIN, WRAP_ROWS), np.float16)
        t[:, :NC_ROWS] = feats[c * NC_ROWS:(c + 1) * NC_ROWS].T
        ftcs.append(t)
    return f4, wm, gbv, ftcs


def kernel(feats, weight, gamma, beta, neighbor_idx):
    from concourse.bass_utils import run_bass_kernel_spmd

    nbr = np.asarray(neighbor_idx)
    meta, gidx_cores, sidx_cores = _plan(nbr)
    nc = _build_bass(meta)
    f4, wm, gbv, ftcs = _host_tensors(feats, weight, gamma, beta)
    in_maps = [
        {"feats4": f4, "wmat": wm, "ftc": ftcs[c], "gidx": gidx_cores[c],
         "sixd": sidx_cores[c], "gbeta": gbv}
        for c in range(NCORES)
    ]
    res = run_bass_kernel_spmd(nc, in_maps, core_ids=list(range(NCORES)))
    out = np.empty((N, COUT), np.float32)
    for c in range(NCORES):
        wrapped = np.empty((128, SLOTS, COUT), np.float32)
        wrapped[:, 0::2, :] = np.asarray(res.results[c]["oute"],
                                         dtype=np.float32)
        wrapped[:, 1::2, :] = np.asarray(res.results[c]["outo"],
                                         dtype=np.float32)
        rows = wrapped.transpose(1, 0, 2).reshape(WRAP_ROWS, COUT)
        out[c * NC_ROWS:(c + 1) * NC_ROWS] = rows[:NC_ROWS]
    return out


# revision 4
# speedup vs baseline: 1.2442x; 1.2442x over previous
"""Sparse Conv3d (3x3x3, torchsparse kmap) + BatchNorm + ReLU on 8 TRN2 NeuronCores.

Strategy (voxel/data parallel, per sharding hint):
  - Output voxels sharded across 8 cores (15000 rows each).
  - feats replicated in DRAM as 4 banks of 30000 rows (+1 zero row each) so
    gather indices fit int16 for the HW dma_gather path. Feature rows are
    stored fp16 padded to 128 channels (256B) so the gather can run in
    TRANSPOSE mode: the gather output is G^T (channels on partitions)
    directly, eliminating the per-chunk PE transpose + DVE copy of the
    previous version.
  - Center offset (k=13) is the identity map: its matmul operand is loaded
    directly from a per-core transposed feats slice (no gather).
  - Off-center offsets: host-compacted valid (src,dst) pairs; device does
    transpose-dma_gather (bank-major) -> matmul lhsT=G^T chunk, rhs=W_k ->
    dma_scatter_add (fp16 CCE) into fp16 SBUF accumulators (parity-split
    even/odd slots). Scatter calls are per-offset so destinations are unique
    within a call (duplicate dsts within one call lose updates on HW; across
    calls they accumulate correctly).
  - BN stats via PE matmuls (ones^T @ X for sums, X^T X diag for
    sum-squares), AllReduce [1,128] across the 8 cores, normalize + ReLU
    on-chip, fp16 output in a row-wrapped layout that the host unwraps.
"""

import sys
import os

for _p in ("/opt/trn_rl_repo", "/root/.axon_site/_ro/trn_rl_repo"):
    if os.path.isdir(_p) and _p not in sys.path:
        sys.path.insert(0, _p)

import numpy as np

N = 120000
CIN = 64
CPAD = 128                      # fp16 channel padding for 256B gather elems
COUT = 64
K = 27
CENTER = 13
EPS = 1e-5
NCORES = 8
NC_ROWS = N // NCORES          # 15000
BANK = 30000
NBANK = 4
ZROW = BANK                     # zero row index within a bank table
SLOTS = 118                     # ceil(15000/128); wrapped rows = 15104
WRAP_ROWS = SLOTS * 128         # 15104
TRASH = WRAP_ROWS - 1           # trash dst row 15103 (slot 117, partition 127)
HGRP = (SLOTS + 1) // 2         # 59 groups per parity


def _wrap16(idx):
    """Wrap an int stream into the [128, n/16] int16 layout dma_gather expects."""
    n = len(idx)
    assert n % 16 == 0
    w = np.ascontiguousarray(idx.reshape(n // 16, 16).T).astype(np.int16)
    return np.tile(w, (8, 1))


def _plan(nbr):
    """Host-side index preprocessing. Returns static chunk metadata (shared
    across cores) and per-core gather/scatter index tensors."""
    offs = [k for k in range(K) if k != CENTER]
    counts = np.zeros((NCORES, K, NBANK), np.int64)
    lists = {}
    for k in offs:
        v = nbr[k]
        for c in range(NCORES):
            seg = v[c * NC_ROWS:(c + 1) * NC_ROWS]
            val = np.nonzero(seg >= 0)[0]
            src = seg[val].astype(np.int64)
            bank = src // BANK
            for b in range(NBANK):
                m = bank == b
                lists[(c, k, b)] = (src[m] - b * BANK, val[m])
                counts[c, k, b] = int(m.sum())
    ckb = -(-counts.max(axis=0) // 128)  # [K, NBANK] chunks, 0 for CENTER row
    ckb[CENTER, :] = 0

    # gather stream: bank-major, offsets ascending inside each bank
    gbase = np.zeros(NBANK, np.int64)   # slot base per bank
    gslot = {}                          # (k, b) -> first gather slot (global)
    pos = 0
    sg_b = []
    for b in range(NBANK):
        gbase[b] = pos
        for k in offs:
            if ckb[k, b]:
                gslot[(k, b)] = pos
                pos += ckb[k, b]
        sg_b.append(pos - gbase[b])
    sg_total = pos

    # scatter stream: offset-major; chunk (k, b, j) -> scatter slot
    spos = {}
    p = 0
    ck_tot = {}
    for k in offs:
        ck = int(ckb[k].sum())
        ck_tot[k] = ck
        spos[k] = p
        p += ck
    ss_total = p

    gidx_cores, sidx_cores = [], []
    for c in range(NCORES):
        gstream = np.full(sg_total * 128, ZROW, np.int64)
        for b in range(NBANK):
            for k in offs:
                if not ckb[k, b]:
                    continue
                loc, _ = lists[(c, k, b)]
                s0 = gslot[(k, b)] * 128
                gstream[s0:s0 + len(loc)] = loc
        sstream = np.full(ss_total * 128, TRASH, np.int64)
        for k in offs:
            base = spos[k] * 128
            o = 0
            for b in range(NBANK):
                if not ckb[k, b]:
                    continue
                _, dst = lists[(c, k, b)]
                sstream[base + o:base + o + len(dst)] = dst
                o += ckb[k, b] * 128
        # wrap per call: gather call = per bank; scatter call = per offset
        gw = [
            _wrap16(gstream[gbase[b] * 128:(gbase[b] + sg_b[b]) * 128])
            for b in range(NBANK) if sg_b[b]
        ]
        sw = [
            _wrap16(sstream[spos[k] * 128:(spos[k] + ck_tot[k]) * 128])
            for k in offs if ck_tot[k]
        ]
        gidx_cores.append(np.concatenate(gw, axis=1))
        sidx_cores.append(np.concatenate(sw, axis=1))

    meta = dict(offs=offs, ckb=ckb, gbase=gbase, sg_b=sg_b, sg_total=sg_total,
                gslot=gslot, spos=spos, ck_tot=ck_tot, ss_total=ss_total)
    return meta, gidx_cores, sidx_cores


def _build_bass(meta, repeat=1):
    from concourse import mybir, bacc
    import concourse.tile as tile
    from concourse.masks import make_identity

    offs = meta["offs"]
    ckb = meta["ckb"]
    gbase = meta["gbase"]
    sg_b = meta["sg_b"]
    gslot = meta["gslot"]
    ck_tot = meta["ck_tot"]
    sg_total = meta["sg_total"]
    ss_total = meta["ss_total"]
    f32 = mybir.dt.float32
    f16 = mybir.dt.float16
    i16 = mybir.dt.int16

    nc = bacc.Bacc("TRN2", target_bir_lowering=False, debug=False,
                   num_devices=NCORES)
    feats4 = nc.dram_tensor("feats4", [NBANK * (BANK + 1), CPAD], f16,
                            kind="ExternalInput").ap()
    wmat = nc.dram_tensor("wmat", [CPAD, K * COUT], f16,
                          kind="ExternalInput").ap()
    ftc = nc.dram_tensor("ftc", [CIN, WRAP_ROWS], f16,
                         kind="ExternalInput").ap()
    gidx = nc.dram_tensor("gidx", [128, sg_total * 8], i16,
                          kind="ExternalInput").ap()
    sixd = nc.dram_tensor("sixd", [128, ss_total * 8], i16,
                          kind="ExternalInput").ap()
    gbeta = nc.dram_tensor("gbeta", [1, 128], f32, kind="ExternalInput").ap()
    oute = nc.dram_tensor("oute", [128, HGRP, COUT], f16,
                          kind="ExternalOutput").ap()
    outo = nc.dram_tensor("outo", [128, HGRP, COUT], f16,
                          kind="ExternalOutput").ap()

    with tile.TileContext(nc) as tc:
        with tc.tile_pool(name="sb", bufs=1) as pool, \
             tc.tile_pool(name="ps", bufs=2, space="PSUM") as psum, \
             tc.tile_pool(name="dram", bufs=1, space="DRAM") as dram:
            for _rep in range(repeat):
                ident = pool.tile([128, 128], f32)
                make_identity(nc, ident[:])
                ones16 = pool.tile([128, 1], f16)
                nc.vector.memset(ones16[:], 1.0)
                onesr = pool.tile([1, 128], f32)
                nc.vector.memset(onesr[:], 1.0)
                istack = pool.tile([128, COUT], f32)
                nc.vector.tensor_copy(out=istack[0:64, :], in_=ident[0:64, 0:64])
                nc.vector.tensor_copy(out=istack[64:128, :],
                                      in_=ident[64:128, 64:128])

                gix = pool.tile([128, sg_total * 8], i16)
                nc.sync.dma_start(out=gix[:], in_=gidx[:])
                six = pool.tile([128, ss_total * 8], i16)
                nc.sync.dma_start(out=six[:], in_=sixd[:])
                wsb = pool.tile([CPAD, K * COUT], f16)
                nc.sync.dma_start(out=wsb[:], in_=wmat[:])
                gb = pool.tile([1, 128], f32)
                nc.sync.dma_start(out=gb[:], in_=gbeta[:])

                # accumulators (SBUF resident, fp16), initialized by center pass
                ae = pool.tile([128, HGRP, COUT], f16)
                ao = pool.tile([128, HGRP, COUT], f16)

                # ---- transpose-mode gathers (bank-major stream) ----
                # Each bank is split into two k-range calls so the first
                # half's matmul+scatter pipeline can start while the second
                # half is still gathering.
                gtiles = {}
                for b in range(NBANK):
                    if not sg_b[b]:
                        continue
                    g = pool.tile([128, 1, sg_b[b] * 128], f16, tag=f"g{b}")
                    gtiles[b] = g
                for half in range(2):
                    for b in range(NBANK):
                        if not sg_b[b]:
                            continue
                        mid = sg_b[b] // 2
                        c0, cn = (0, mid) if half == 0 else (mid, sg_b[b] - mid)
                        if not cn:
                            continue
                        g = gtiles[b]
                        nc.gpsimd.dma_gather(
                            out_ap=g[:, :, c0 * 128:(c0 + cn) * 128],
                            in_ap=feats4[(BANK + 1) * b:(BANK + 1) * (b + 1), :],
                            idxs_ap=gix[:, (gbase[b] + c0) * 8:
                                        (gbase[b] + c0 + cn) * 8],
                            num_idxs=cn * 128, num_idxs_reg=cn * 128,
                            elem_size=CPAD, transpose=True,
                            single_packet=False)

                # ---- center pass: ftc slices -> matmul -> init ae/ao ----
                piece_chunks = [30, 30, 30, 28]
                wc = wsb[0:CIN, CENTER * COUT:(CENTER + 1) * COUT]
                jglob = 0
                for pc_i, pch in enumerate(piece_chunks):
                    fpc = pool.tile([CIN, pch * 128], f16, tag="ftc", bufs=2)
                    nc.sync.dma_start(
                        out=fpc[:], in_=ftc[:, jglob * 128:(jglob + pch) * 128])
                    for j0 in range(0, pch, 8):
                        jn = min(8, pch - j0)
                        pe = psum.tile([128, 4, COUT], f32, tag="pcE")
                        po = psum.tile([128, 4, COUT], f32, tag="pcO")
                        ne = no = 0
                        for j in range(j0, j0 + jn):
                            gj = jglob + j
                            lhsT = fpc[:, j * 128:(j + 1) * 128]
                            if gj % 2 == 0:
                                out_ap = pe[:, ne, :]; ne += 1
                            else:
                                out_ap = po[:, no, :]; no += 1
                            nc.tensor.matmul(out=out_ap, lhsT=lhsT, rhs=wc,
                                             start=True, stop=True)
                        ge0 = (jglob + j0) // 2
                        go0 = (jglob + j0) // 2
                        if ne:
                            nc.vector.tensor_copy(out=ae[:, ge0:ge0 + ne, :],
                                                  in_=pe[:, :ne, :])
                        if no:
                            nc.vector.tensor_copy(out=ao[:, go0:go0 + no, :],
                                                  in_=po[:, :no, :])
                    jglob += pch

                # ---- off-center: G^T chunk matmul -> scatter-add ----
                for k in offs:
                    ck = ck_tot[k]
                    if not ck:
                        continue
                    y = pool.tile([128, ck, COUT], f16, tag="y", bufs=3)
                    wk = wsb[:, k * COUT:(k + 1) * COUT]
                    # chunk list for this offset in scatter order
                    chunks = []
                    for b in range(NBANK):
                        for j in range(ckb[k, b]):
                            chunks.append(gslot[(k, b)] - gbase[b] + j
                                          + (b << 20))  # encode bank
                    for i0 in range(0, ck, 8):
                        inb = min(8, ck - i0)
                        py = psum.tile([128, 8, COUT], f32, tag="py")
                        for q in range(inb):
                            enc = chunks[i0 + q]
                            b, slot = enc >> 20, enc & ((1 << 20) - 1)
                            gT = gtiles[b][:, 0, slot * 128:(slot + 1) * 128]
                            nc.tensor.matmul(out=py[:, q, :], lhsT=gT, rhs=wk,
                                             start=True, stop=True)
                        nc.vector.tensor_copy(out=y[:, i0:i0 + inb, :],
                                              in_=py[:, :inb, :])
                    nc.gpsimd.dma_scatter_add(
                        out_ap=ae[:], in_ap=y[:, :, :],
                        idxs_ap=six[:, meta["spos"][k] * 8:(meta["spos"][k] + ck) * 8],
                        num_idxs=ck * 128, num_idxs_reg=ck * 128, elem_size=COUT,
                        sbuf_tokens_per_rank=128, parity_reg=0, out_ap_other=ao[:],
                        single_packet=False)

                # ---- zero the trash region (rows 15072..15103 incl. TRASH) ----
                # other pad rows (15000..15071) only ever receive center zeros
                nc.vector.memset(ao[96:128, 58, :], 0.0)

                # ---- stats: sums + sum-squares over all rows ----
                pcov = psum.tile([128, 128], f32, tag="py")
                cov_ins = []
                for t in (ae, ao):
                    for g0 in range(0, HGRP - 1, 2):
                        cov_ins.append(t[:, g0:g0 + 2, :])
                    cov_ins.append(t[:, HGRP - 1:HGRP, :])
                for i, ap in enumerate(cov_ins):
                    w = ap.shape[1] * COUT
                    nc.tensor.matmul(out=pcov[0:w, 0:w], lhsT=ap, rhs=ap,
                                     start=(i == 0), stop=(i == len(cov_ins) - 1))
                psumr = psum.tile([1, 512], f32, tag="pcE")
                sum_ins = []
                for t in (ae, ao):
                    for g0 in range(0, HGRP, 8):
                        gn = min(8, HGRP - g0)
                        sum_ins.append(t[:, g0:g0 + gn, :])
                for i, ap in enumerate(sum_ins):
                    w = ap.shape[1] * COUT
                    nc.tensor.matmul(out=psumr[:, 0:w], lhsT=ones16[:], rhs=ap,
                                     start=(i == 0), stop=(i == len(sum_ins) - 1))
                tmpc = pool.tile([128, 128], f32)
                nc.vector.tensor_mul(out=tmpc[:], in0=pcov[:], in1=ident[:])
                diagc = pool.tile([128, 1], f32)
                nc.vector.tensor_reduce(out=diagc[:], in_=tmpc[:],
                                        axis=mybir.AxisListType.X,
                                        op=mybir.AluOpType.add)
                psq = psum.tile([1, COUT], f32, tag="pt")
                nc.tensor.matmul(out=psq[:], lhsT=diagc[:], rhs=istack[:],
                                 start=True, stop=True)
                ssum = pool.tile([1, 512], f32)
                nc.vector.tensor_copy(out=ssum[:], in_=psumr[:])
                nc.vector.tensor_add(out=ssum[:, 0:256], in0=ssum[:, 0:256],
                                     in1=ssum[:, 256:512])
                nc.vector.tensor_add(out=ssum[:, 0:128], in0=ssum[:, 0:128],
                                     in1=ssum[:, 128:256])
                nc.vector.tensor_add(out=ssum[:, 0:64], in0=ssum[:, 0:64],
                                     in1=ssum[:, 64:128])
                stats = pool.tile([1, 128], f32)
                nc.vector.tensor_copy(out=stats[:, 0:64], in_=ssum[:, 0:64])
                nc.vector.tensor_copy(out=stats[:, 64:128], in_=psq[:])

                # ---- AllReduce over 8 cores ----
                cin_d = dram.tile([1, 128], f32)
                cout_d = dram.tile([1, 128], f32)
                nc.sync.dma_start(out=cin_d[:], in_=stats[:])
                nc.gpsimd.collective_compute(
                    "AllReduce", mybir.AluOpType.add,
                    replica_groups=[list(range(NCORES))],
                    ins=[cin_d.opt()], outs=[cout_d.opt()])
                red = pool.tile([1, 128], f32)
                nc.sync.dma_start(out=red[:], in_=cout_d[:])

                # ---- affine params ----
                mean = pool.tile([1, COUT], f32)
                nc.vector.tensor_scalar_mul(out=mean[:], in0=red[:, 0:64],
                                            scalar1=1.0 / N)
                ex2 = pool.tile([1, COUT], f32)
                nc.vector.tensor_scalar_mul(out=ex2[:], in0=red[:, 64:128],
                                            scalar1=1.0 / N)
                var = pool.tile([1, COUT], f32)
                nc.vector.tensor_mul(out=var[:], in0=mean[:], in1=mean[:])
                nc.vector.tensor_sub(out=var[:], in0=ex2[:], in1=var[:])
                nc.vector.tensor_scalar_add(out=var[:], in0=var[:], scalar1=EPS)
                std = pool.tile([1, COUT], f32)
                nc.scalar.sqrt(out=std[:], in_=var[:])
                rstd = pool.tile([1, COUT], f32)
                nc.vector.reciprocal(out=rstd[:], in_=std[:])
                scl = pool.tile([1, COUT], f32)
                nc.vector.tensor_mul(out=scl[:], in0=gb[:, 0:64], in1=rstd[:])
                bia = pool.tile([1, COUT], f32)
                nc.vector.tensor_mul(out=bia[:], in0=mean[:], in1=scl[:])
                nc.vector.tensor_sub(out=bia[:], in0=gb[:, 64:128], in1=bia[:])

                # broadcast to [128, 8, 64] fp16 (pattern repeats every 64 cols)
                pbs = psum.tile([128, COUT], f32, tag="pt")
                nc.tensor.matmul(out=pbs[:], lhsT=onesr[:], rhs=scl[:],
                                 start=True, stop=True)
                s8 = pool.tile([128, 8, COUT], f16)
                nc.vector.tensor_copy(out=s8[:, 0, :], in_=pbs[:])
                pbb = psum.tile([128, COUT], f32, tag="pt")
                nc.tensor.matmul(out=pbb[:], lhsT=onesr[:], rhs=bia[:],
                                 start=True, stop=True)
                b8 = pool.tile([128, 8, COUT], f16)
                nc.vector.tensor_copy(out=b8[:, 0, :], in_=pbb[:])
                for t8 in (s8, b8):
                    nc.vector.tensor_copy(out=t8[:, 1:2, :], in_=t8[:, 0:1, :])
                    nc.vector.tensor_copy(out=t8[:, 2:4, :], in_=t8[:, 0:2, :])
                    nc.vector.tensor_copy(out=t8[:, 4:8, :], in_=t8[:, 0:4, :])

                # ---- normalize + relu in place, then write out ----
                for t in (ae, ao):
                    for g0 in range(0, HGRP, 8):
                        gn = min(8, HGRP - g0)
                        sl = t[:, g0:g0 + gn, :]
                        nc.vector.tensor_mul(out=sl, in0=sl, in1=s8[:, 0:gn, :])
                        nc.vector.tensor_add(out=sl, in0=sl, in1=b8[:, 0:gn, :])
                        nc.vector.tensor_scalar_max(out=sl, in0=sl, scalar1=0.0)
                nc.sync.dma_start(out=oute[:], in_=ae[:, :, :])
                nc.sync.dma_start(out=outo[:], in_=ao[:, :, :])

    nc.compile()
    return nc


def _host_tensors(feats, weight, gamma, beta):
    feats = np.ascontiguousarray(np.asarray(feats, dtype=np.float32))
    weight = np.asarray(weight, dtype=np.float32)
    f4 = np.zeros((NBANK * (BANK + 1), CPAD), np.float16)
    for b in range(NBANK):
        f4[b * (BANK + 1):b * (BANK + 1) + BANK, :CIN] = \
            feats[b * BANK:(b + 1) * BANK]
    wm = np.zeros((CPAD, K * COUT), np.float16)
    wm[:CIN, :] = weight.transpose(1, 0, 2).reshape(CIN, K * COUT)
    gbv = np.zeros((1, 128), np.float32)
    gbv[0, 0:64] = np.asarray(gamma, np.float32)
    gbv[0, 64:128] = np.asarray(beta, np.float32)
    ftcs = []
    for c in range(NCORES):
        t = np.zeros((CIN, WRAP_ROWS), np.float16)
        t[:, :NC_ROWS] = feats[c * NC_ROWS:(c + 1) * NC_ROWS].T
        ftcs.append(t)
    return f4, wm, gbv, ftcs


def kernel(feats, weight, gamma, beta, neighbor_idx):
    from concourse.bass_utils import run_bass_kernel_spmd

    nbr = np.asarray(neighbor_idx)
    meta, gidx_cores, sidx_cores = _plan(nbr)
    nc = _build_bass(meta)
    f4, wm, gbv, ftcs = _host_tensors(feats, weight, gamma, beta)
    in_maps = [
        {"feats4": f4, "wmat": wm, "ftc": ftcs[c], "gidx": gidx_cores[c],
         "sixd": sidx_cores[c], "gbeta": gbv}
        for c in range(NCORES)
    ]
    res = run_bass_kernel_spmd(nc, in_maps, core_ids=list(range(NCORES)))
    out = np.empty((N, COUT), np.float32)
    for c in range(NCORES):
        wrapped = np.empty((128, SLOTS, COUT), np.float32)
        wrapped[:, 0::2, :] = np.asarray(res.results[c]["oute"],
                                         dtype=np.float32)
        wrapped[:, 1::2, :] = np.asarray(res.results[c]["outo"],
                                         dtype=np.float32)
        rows = wrapped.transpose(1, 0, 2).reshape(WRAP_ROWS, COUT)
        out[c * NC_ROWS:(c + 1) * NC_ROWS] = rows[:NC_ROWS]
    return out
